# revision 10
# baseline (speedup 1.0000x reference)
"""Trainium2 Bass kernel for nn_MultiHeadGroupAttn.

Math refactor (vs reference):
  Q  = q_src @ Wq.T                                   [B, NH, HD]
  Qk[b,h,:] = Q[b,h,:] @ Wk[h*HD:(h+1)*HD, :]         (per-head fold of Wk)
  logits[b,h,n] = (Qk[b,h,:] . group[b,n,:]) * srow[b] + maskadd[b,n]
  attn = softmax(logits)                              (exact match w/ ref)
  G[b,h,:] = sum_n attn[b,h,n] * group[b,n,:]
  ctx[b,h,d] = G[b,h,:] . Wv[h*HD+d, :]
  out = ctx @ Wo.T
This avoids materializing K/V (cuts FLOPs ~20x) and reads `group` from HBM
exactly once per core.

Sharding: data-parallel over B across 8 NeuronCores (256 rows each), weights
replicated, no collectives.
"""

import math
from contextlib import ExitStack

import numpy as np
import ml_dtypes

import concourse.bass as bass
from concourse import bacc
import concourse.mybir as mybir
import concourse.tile as tile
from concourse.bass_utils import run_bass_kernel_spmd
from concourse.masks import make_identity

F32 = mybir.dt.float32
BF16 = mybir.dt.bfloat16

B, N, H, NH, HD = 2048, 64, 1024, 16, 64
NCORES = 8
P = 128
HT = H // P  # 8

# attention-core dtype: BF16 (fast, ~0.3% rel err) or F32 (slow, accurate)
ADT = BF16


def _np_dt(adt):
    return ml_dtypes.bfloat16 if adt == BF16 else np.float32


def build_program(nsub=4, adt=ADT, finalize=True):
    """One-core program; nsub*64 batch rows per core (nsub=4 -> 256)."""
    BC = nsub * 64            # batch rows this core
    NPAIR = 32                # pairs per sub (2 rows each)
    NGRP = 8                  # 8-row groups per sub
    NG = BC // 8              # 8-row groups whole core

    nc = bacc.Bacc()

    d_qsrcT = nc.dram_tensor("qsrcT", [H, BC], adt, kind="ExternalInput")
    d_grp = nc.dram_tensor("grp", [BC * N, H], adt, kind="ExternalInput")
    d_wqT = nc.dram_tensor("wqT", [H, H], adt, kind="ExternalInput")
    d_wk = nc.dram_tensor("wk", [H, H], adt, kind="ExternalInput")
    d_wvT = nc.dram_tensor("wvT", [H, H], adt, kind="ExternalInput")
    d_woT = nc.dram_tensor("woT", [H, H], F32, kind="ExternalInput")
    d_mask = nc.dram_tensor("maskadd", [BC * NH, N], F32, kind="ExternalInput")
    d_srow = nc.dram_tensor("srow", [BC * NH, 1], F32, kind="ExternalInput")
    d_outT = nc.dram_tensor("outT", [H, BC], F32, kind="ExternalOutput")
    d_attn = nc.dram_tensor("attn", [BC * NH, N], F32, kind="ExternalOutput")

    with tile.TileContext(nc) as tc:
        with ExitStack() as ctx:
            wp = ctx.enter_context(tc.tile_pool(name="wp", bufs=1))
            main = ctx.enter_context(tc.tile_pool(name="main", bufs=1))
            work = ctx.enter_context(tc.tile_pool(name="work", bufs=1))
            ps = ctx.enter_context(tc.tile_pool(name="ps", bufs=1, space="PSUM"))

            ident = wp.tile([P, P], adt, tag="ident")
            make_identity(nc, ident)

            wk_sb, wvT_sb, woT_sb = [], [], []
            for t in range(HT):
                wkt = wp.tile([P, H], adt, tag=f"wk{t}")
                nc.sync.dma_start(out=wkt, in_=d_wk[t * P:(t + 1) * P, :])
                wk_sb.append(wkt)
                wvt = wp.tile([P, H], adt, tag=f"wvT{t}")
                nc.sync.dma_start(out=wvt, in_=d_wvT[t * P:(t + 1) * P, :])
                wvT_sb.append(wvt)
                wot = wp.tile([P, H], F32, tag=f"woT{t}")
                nc.sync.dma_start(out=wot, in_=d_woT[t * P:(t + 1) * P, :])
                woT_sb.append(wot)

            mask_sb = wp.tile([P, NG, N], F32, tag="mask")
            nc.sync.dma_start(
                out=mask_sb, in_=d_mask.rearrange("(g p) n -> p g n", p=P))
            srow_sb = wp.tile([P, NG], F32, tag="srow")
            nc.sync.dma_start(
                out=srow_sb, in_=d_srow.rearrange("(g p) x -> p (g x)", p=P))

            # ---------- phase: QT[o, b] = Wq @ q_src^T ----------
            QT_sb = [wp.tile([P, BC], adt, tag=f"QT{t}", name=f"QT{t}") for t in range(HT)]
            with tc.tile_pool(name="qphase", bufs=1) as qp:
                wq_sb, qs_sb = [], []
                for t in range(HT):
                    wqt = qp.tile([P, H], adt, tag=f"wqT{t}")
                    nc.sync.dma_start(out=wqt, in_=d_wqT[t * P:(t + 1) * P, :])
                    wq_sb.append(wqt)
                    qst = qp.tile([P, BC], adt, tag=f"qs{t}")
                    nc.sync.dma_start(out=qst, in_=d_qsrcT[t * P:(t + 1) * P, :])
                    qs_sb.append(qst)
                for ot in range(HT):
                    pq = ps.tile([P, BC], F32, tag="pq", bufs=2)
                    for jc in range(HT):
                        nc.tensor.matmul(
                            pq, wq_sb[jc][:, ot * P:(ot + 1) * P], qs_sb[jc],
                            start=(jc == 0), stop=(jc == HT - 1))
                    if ot % 2 == 0:
                        nc.vector.tensor_copy(out=QT_sb[ot], in_=pq)
                    else:
                        nc.scalar.copy(out=QT_sb[ot], in_=pq)

            # Zero-padded block-diag Qk^T tiles (persistent, parity-buffered):
            # qz0[par][jt] cols p*32+[0:16] = Qk^T of even row of pair p,
            #              cols p*32+[16:32] = zeros; qz1 is the mirror.
            qz0 = [wp.tile([P, 32 * 32], adt, tag=f"qz0_{t}",
                           name=f"qz0_{t}") for t in range(HT)]
            qz1 = [wp.tile([P, 32 * 32], adt, tag=f"qz1_{t}",
                           name=f"qz1_{t}") for t in range(HT)]
            for t in range(HT):
                nc.gpsimd.memset(qz0[t], 0.0)
                nc.gpsimd.memset(qz1[t], 0.0)

            # ---------- main loop over subs (64 rows each) ----------
            for s in range(nsub):
                # Qk this sub, written into the nonzero halves of qz0/qz1
                for h in range(NH):
                    off = (h % 2) * 64
                    wt = h // 2
                    for jt in range(HT):
                        pqk = ps.tile([P, 64], F32, tag="pq", bufs=2)
                        nc.tensor.matmul(
                            pqk,
                            wk_sb[wt][off:off + 64, jt * P:(jt + 1) * P],
                            QT_sb[wt][off:off + 64, s * 64:(s + 1) * 64],
                            start=True, stop=True)
                        src = pqk.rearrange("p (pr e) -> p pr e", e=2)
                        d0 = qz0[jt].rearrange(
                            "p (pr c) -> p pr c", c=32)[:, :, h:h + 1]
                        d1 = qz1[jt].rearrange(
                            "p (pr c) -> p pr c", c=32)[:, :, 16 + h:17 + h]
                        if (h + jt) % 2 == 0:
                            nc.vector.tensor_copy(out=d0, in_=src[:, :, 0:1])
                            nc.scalar.copy(out=d1, in_=src[:, :, 1:2])
                        else:
                            nc.scalar.copy(out=d0, in_=src[:, :, 0:1])
                            nc.vector.tensor_copy(out=d1, in_=src[:, :, 1:2])

                # G^T accumulator layout gts[jt][j, h*64+b_local]
                gts = [main.tile([P, NH * 64], adt, tag=f"gts{t}", name=f"gts{t}", bufs=2)
                       for t in range(HT)]

                for grp_i in range(NGRP):
                    g = s * NGRP + grp_i      # global 8-row group index
                    gtiles = [None, None]     # two 2-pair loads per group
                    stage = work.tile([P, N], F32, tag="stage", bufs=2)

                    for q in range(4):        # 4 pairs in the group
                        p_ = grp_i * 4 + q    # pair index within sub
                        if q % 2 == 0:
                            gt_ = work.tile([P, 2, H], adt, tag="grp", bufs=6)
                            base = s * (64 * N) + (p_ // 2) * 256
                            nc.sync.dma_start(
                                out=gt_,
                                in_=d_grp[base:base + 256, :].rearrange(
                                    "(ph p) j -> p ph j", p=P))
                            gtiles[q // 2] = gt_
                        gtile = gtiles[q // 2]
                        ph = q % 2

                        lg = ps.tile([32, N], F32, tag="lg", bufs=1)
                        for jt in range(HT):
                            pt = ps.tile([P, P], adt, tag="pt", bufs=2)
                            nc.tensor.transpose(
                                pt, gtile[:, ph, jt * P:(jt + 1) * P], ident)
                            gT = work.tile([P, P], adt, tag="gT", bufs=4)
                            if jt % 2 == 0:
                                nc.vector.tensor_copy(out=gT, in_=pt)
                            else:
                                nc.scalar.copy(out=gT, in_=pt)
                            cs = p_ * 32
                            nc.tensor.matmul(
                                lg, qz0[jt][:, cs:cs + 32], gT[:, 0:64],
                                start=(jt == 0), stop=False)
                            nc.tensor.matmul(
                                lg, qz1[jt][:, cs:cs + 32], gT[:, 64:128],
                                start=False, stop=(jt == HT - 1))

                        # scale + mask into stage rows (whole 32-row pair)
                        r0 = q * 32
                        nc.vector.scalar_tensor_tensor(
                            out=stage[r0:r0 + 32, :], in0=lg,
                            scalar=srow_sb[r0:r0 + 32, g:g + 1],
                            in1=mask_sb[r0:r0 + 32, g, :],
                            op0=mybir.AluOpType.mult, op1=mybir.AluOpType.add)

                    # softmax over the whole 8-row group [128, 64]
                    gg = grp_i % 4
                    if gg == 0:
                        attn_st = work.tile([P, 4, N], F32, tag="attn_st",
                                            bufs=2)
                    negmx = work.tile([P, 1], F32, tag="negmx", bufs=2)
                    nc.vector.tensor_reduce(
                        out=negmx, in_=stage, axis=mybir.AxisListType.X,
                        op=mybir.AluOpType.max, negate=True)
                    ex = work.tile([P, N], F32, tag="ex", bufs=2)
                    ssum = work.tile([P, 1], F32, tag="ssum", bufs=2)
                    nc.scalar.activation(
                        out=ex, in_=stage,
                        func=mybir.ActivationFunctionType.Exp,
                        bias=negmx, scale=1.0, accum_out=ssum)
                    rr = work.tile([P, 1], F32, tag="rr", bufs=2)
                    nc.vector.reciprocal(out=rr, in_=ssum)
                    nc.vector.tensor_scalar_mul(attn_st[:, gg, :], ex, rr)
                    att16 = work.tile([P, N], adt, tag="att16", bufs=2)
                    nc.scalar.mul(att16, ex, rr)
                    if gg == 3:
                        gb = (g - 3) // 4
                        nc.sync.dma_start(
                            out=d_attn.rearrange(
                                "(g p) n -> p g n", p=P)[:, 4 * gb:4 * gb + 4, :],
                            in_=attn_st)

                    # attn^T of group: patful[0:64]=T(att16); dup to [64:128]
                    pat = ps.tile([64, P], adt, tag="pt", bufs=2)
                    nc.tensor.transpose(pat, att16, ident)
                    patful = work.tile([P, P], adt, tag="patful", bufs=2)
                    nc.vector.tensor_copy(out=patful[0:64, :], in_=pat)
                    nc.sync.dma_start(out=patful[64:128, :],
                                      in_=patful[0:64, :])

                    # G pass for the 4 pairs of this group
                    for q in range(4):
                        p_ = grp_i * 4 + q
                        gtile = gtiles[q // 2]
                        ph = q % 2
                        abd = work.tile([P, 32], adt, tag="abd", bufs=2)
                        nc.gpsimd.memset(abd, 0.0)
                        c0 = (2 * q) * 16
                        c1 = (2 * q + 1) * 16
                        nc.vector.tensor_copy(
                            out=abd[0:64, 0:16], in_=patful[0:64, c0:c0 + 16])
                        nc.vector.tensor_copy(
                            out=abd[64:128, 16:32],
                            in_=patful[64:128, c1:c1 + 16])
                        for jt in range(HT):
                            pg = ps.tile([P, 32], F32, tag="pg", bufs=2)
                            nc.tensor.matmul(
                                pg, gtile[:, ph, jt * P:(jt + 1) * P], abd,
                                start=True, stop=True)
                            dst = gts[jt].rearrange(
                                "p (h b) -> p b h", b=64)[:, 2 * p_:2 * p_ + 2, :]
                            src = pg.rearrange("p (b h) -> p b h", h=16)
                            if jt % 2 == 0:
                                nc.scalar.copy(out=dst, in_=src)
                            else:
                                nc.vector.tensor_copy(out=dst, in_=src)

                # ---- ctx per head: ctxT[h//2][(h%2)*64+d, b] ----
                ctxT = [main.tile([P, 64], F32, tag=f"ctxT{t}", name=f"ctxT{t}", bufs=2)
                        for t in range(HT)]
                for h in range(NH):
                    pc = ps.tile([P, 64], F32, tag="pc", bufs=1)
                    for jc in range(HT):
                        nc.tensor.matmul(
                            pc,
                            wvT_sb[jc][:, (h // 2) * P:(h // 2 + 1) * P],
                            gts[jc][:, h * 64:(h + 1) * 64],
                            start=(jc == 0), stop=(jc == HT - 1))
                    off = (h % 2) * 64
                    if h % 2 == 0:
                        nc.vector.tensor_copy(
                            out=ctxT[h // 2][off:off + 64, :],
                            in_=pc[off:off + 64, :])
                    else:
                        nc.scalar.copy(
                            out=ctxT[h // 2][off:off + 64, :],
                            in_=pc[off:off + 64, :])

                # ---- out projection ----
                ostg = work.tile([P, HT, 64], F32, tag="ostg", bufs=2)
                for ot in range(HT):
                    po = ps.tile([P, 64], F32, tag="pc", bufs=1)
                    for oc in range(HT):
                        nc.tensor.matmul(
                            po, woT_sb[oc][:, ot * P:(ot + 1) * P], ctxT[oc],
                            start=(oc == 0), stop=(oc == HT - 1))
                    if ot % 2 == 0:
                        nc.vector.tensor_copy(out=ostg[:, ot, :], in_=po)
                    else:
                        nc.scalar.copy(out=ostg[:, ot, :], in_=po)
                nc.sync.dma_start(
                    out=d_outT.rearrange(
                        "(t p) b -> p t b", p=P)[:, :, s * 64:(s + 1) * 64],
                    in_=ostg)

    if finalize:
        nc.finalize()
    return nc


def host_prep(q_src, group, mask, group_temp, Wq, Wk, Wv, Wo, nsub=4, adt=ADT):
    """Build per-core in_maps from full inputs."""
    npdt = _np_dt(adt)
    BC = nsub * 64
    wqT = np.ascontiguousarray(Wq.T).astype(npdt)
    wk = Wk.astype(npdt)
    wvT = np.ascontiguousarray(Wv.T).astype(npdt)
    woT = np.ascontiguousarray(Wo.T).astype(np.float32)
    srow_full = (1.0 / (math.sqrt(HD) * group_temp.astype(np.float64))).astype(
        np.float32)                                           # [B]
    maskadd_full = np.where(mask, 0.0, -1e9).astype(np.float32)  # [B, N]

    in_maps = []
    for c in range(NCORES):
        r0 = c * BC
        rows = slice(r0, r0 + BC)
        qsrcT = np.ascontiguousarray(q_src[rows].T).astype(npdt)
        grp = np.ascontiguousarray(
            group[rows].reshape(BC * N, H)).astype(npdt)
        ma = np.repeat(maskadd_full[rows, None, :], NH, axis=1).reshape(
            BC * NH, N).astype(np.float32)
        sr = np.repeat(srow_full[rows, None], NH, axis=1).reshape(
            BC * NH, 1).astype(np.float32)
        in_maps.append({
            "qsrcT": qsrcT, "grp": grp, "wqT": wqT, "wk": wk,
            "wvT": wvT, "woT": woT, "maskadd": ma, "srow": sr,
        })
    return in_maps


_prog_cache = {}


def kernel(q_src, group, mask, group_temp, Wq, Wk, Wv, Wo):
    q_src = np.asarray(q_src, dtype=np.float32)
    group = np.asarray(group, dtype=np.float32)
    mask = np.asarray(mask)
    group_temp = np.asarray(group_temp, dtype=np.float32)
    Wq = np.asarray(Wq, dtype=np.float32)
    Wk = np.asarray(Wk, dtype=np.float32)
    Wv = np.asarray(Wv, dtype=np.float32)
    Wo = np.asarray(Wo, dtype=np.float32)

    key = ("prog", 4, str(ADT))
    if key not in _prog_cache:
        _prog_cache[key] = build_program(nsub=4, adt=ADT)
    nc = _prog_cache[key]

    in_maps = host_prep(q_src, group, mask, group_temp, Wq, Wk, Wv, Wo)
    res = run_bass_kernel_spmd(nc, in_maps, core_ids=list(range(NCORES)))

    out = np.concatenate(
        [res.results[c]["outT"].T for c in range(NCORES)], axis=0)
    attn = np.concatenate(
        [res.results[c]["attn"].reshape(256, NH, N) for c in range(NCORES)],
        axis=0)
    return out.astype(np.float32), attn.astype(np.float32)


# revision 13
# speedup vs baseline: 1.0013x; 1.0013x over previous
"""Trainium2 Bass kernel for nn_MultiHeadGroupAttn.

Math refactor (vs reference):
  Q  = q_src @ Wq.T                                   [B, NH, HD]
  Qk[b,h,:] = Q[b,h,:] @ Wk[h*HD:(h+1)*HD, :]         (per-head fold of Wk)
  logits[b,h,n] = (Qk[b,h,:] . group[b,n,:]) * srow[b] + maskadd[b,n]
  attn = softmax(logits)                              (exact match w/ ref)
  G[b,h,:] = sum_n attn[b,h,n] * group[b,n,:]
  ctx[b,h,d] = G[b,h,:] . Wv[h*HD+d, :]
  out = ctx @ Wo.T
This avoids materializing K/V (cuts FLOPs ~20x) and reads `group` from HBM
exactly once per core.

Sharding: data-parallel over B across 8 NeuronCores (256 rows each), weights
replicated, no collectives.
"""

import math
from contextlib import ExitStack

import numpy as np
import ml_dtypes

import concourse.bass as bass
from concourse import bacc
import concourse.mybir as mybir
import concourse.tile as tile
from concourse.bass_utils import run_bass_kernel_spmd
from concourse.masks import make_identity

F32 = mybir.dt.float32
BF16 = mybir.dt.bfloat16

B, N, H, NH, HD = 2048, 64, 1024, 16, 64
NCORES = 8
P = 128
HT = H // P  # 8

# attention-core dtype: BF16 (fast, ~0.3% rel err) or F32 (slow, accurate)
ADT = BF16


def _np_dt(adt):
    return ml_dtypes.bfloat16 if adt == BF16 else np.float32


def build_program(nsub=4, adt=ADT, finalize=True):
    """One-core program; nsub*64 batch rows per core (nsub=4 -> 256)."""
    BC = nsub * 64            # batch rows this core
    NPAIR = 32                # pairs per sub (2 rows each)
    NGRP = 8                  # 8-row groups per sub
    NG = BC // 8              # 8-row groups whole core

    nc = bacc.Bacc()

    d_qsrcT = nc.dram_tensor("qsrcT", [H, BC], adt, kind="ExternalInput")
    d_grp = nc.dram_tensor("grp", [BC * N, H], adt, kind="ExternalInput")
    d_wqT = nc.dram_tensor("wqT", [H, H], adt, kind="ExternalInput")
    d_wk = nc.dram_tensor("wk", [H, H], adt, kind="ExternalInput")
    d_wvT = nc.dram_tensor("wvT", [H, H], adt, kind="ExternalInput")
    d_woT = nc.dram_tensor("woT", [H, H], F32, kind="ExternalInput")
    d_mask = nc.dram_tensor("maskadd", [BC * NH, N], F32, kind="ExternalInput")
    d_srow = nc.dram_tensor("srow", [BC * NH, 1], F32, kind="ExternalInput")
    d_outT = nc.dram_tensor("outT", [H, BC], F32, kind="ExternalOutput")
    d_attn = nc.dram_tensor("attn", [BC * NH, N], F32, kind="ExternalOutput")

    with tile.TileContext(nc) as tc:
        with ExitStack() as ctx:
            wp = ctx.enter_context(tc.tile_pool(name="wp", bufs=1))
            main = ctx.enter_context(tc.tile_pool(name="main", bufs=1))
            work = ctx.enter_context(tc.tile_pool(name="work", bufs=1))
            ps = ctx.enter_context(tc.tile_pool(name="ps", bufs=1, space="PSUM"))

            ident = wp.tile([P, P], adt, tag="ident")
            make_identity(nc, ident)

            wk_sb, wvT_sb, woT_sb = [], [], []
            for t in range(HT):
                wkt = wp.tile([P, H], adt, tag=f"wk{t}")
                nc.sync.dma_start(out=wkt, in_=d_wk[t * P:(t + 1) * P, :])
                wk_sb.append(wkt)
                wvt = wp.tile([P, H], adt, tag=f"wvT{t}")
                nc.sync.dma_start(out=wvt, in_=d_wvT[t * P:(t + 1) * P, :])
                wvT_sb.append(wvt)
                wot = wp.tile([P, H], F32, tag=f"woT{t}")
                nc.sync.dma_start(out=wot, in_=d_woT[t * P:(t + 1) * P, :])
                woT_sb.append(wot)

            mask_sb = wp.tile([P, NG, N], F32, tag="mask")
            nc.sync.dma_start(
                out=mask_sb, in_=d_mask.rearrange("(g p) n -> p g n", p=P))
            srow_sb = wp.tile([P, NG], F32, tag="srow")
            nc.sync.dma_start(
                out=srow_sb, in_=d_srow.rearrange("(g p) x -> p (g x)", p=P))

            # ---------- phase: QT[o, b] = Wq @ q_src^T ----------
            QT_sb = [wp.tile([P, BC], adt, tag=f"QT{t}", name=f"QT{t}") for t in range(HT)]
            with tc.tile_pool(name="qphase", bufs=1) as qp:
                wq_sb, qs_sb = [], []
                for t in range(HT):
                    wqt = qp.tile([P, H], adt, tag=f"wqT{t}")
                    nc.sync.dma_start(out=wqt, in_=d_wqT[t * P:(t + 1) * P, :])
                    wq_sb.append(wqt)
                    qst = qp.tile([P, BC], adt, tag=f"qs{t}")
                    nc.sync.dma_start(out=qst, in_=d_qsrcT[t * P:(t + 1) * P, :])
                    qs_sb.append(qst)
                for ot in range(HT):
                    pq = ps.tile([P, BC], F32, tag="pq", bufs=2)
                    for jc in range(HT):
                        nc.tensor.matmul(
                            pq, wq_sb[jc][:, ot * P:(ot + 1) * P], qs_sb[jc],
                            start=(jc == 0), stop=(jc == HT - 1))
                    if ot % 2 == 0:
                        nc.vector.tensor_copy(out=QT_sb[ot], in_=pq)
                    else:
                        nc.scalar.copy(out=QT_sb[ot], in_=pq)

            # Zero-padded block-diag Qk^T tiles (persistent, parity-buffered):
            # qz0[par][jt] cols p*32+[0:16] = Qk^T of even row of pair p,
            #              cols p*32+[16:32] = zeros; qz1 is the mirror.
            qz0 = [wp.tile([P, 32 * 32], adt, tag=f"qz0_{t}",
                           name=f"qz0_{t}") for t in range(HT)]
            qz1 = [wp.tile([P, 32 * 32], adt, tag=f"qz1_{t}",
                           name=f"qz1_{t}") for t in range(HT)]
            for t in range(HT):
                nc.gpsimd.memset(qz0[t], 0.0)
                nc.gpsimd.memset(qz1[t], 0.0)

            # ---------- main loop over subs (64 rows each) ----------
            for s in range(nsub):
                # Qk this sub, written into the nonzero halves of qz0/qz1
                for h in range(NH):
                    off = (h % 2) * 64
                    wt = h // 2
                    for jt in range(HT):
                        pqk = ps.tile([P, 64], F32, tag="pq", bufs=2)
                        nc.tensor.matmul(
                            pqk,
                            wk_sb[wt][off:off + 64, jt * P:(jt + 1) * P],
                            QT_sb[wt][off:off + 64, s * 64:(s + 1) * 64],
                            start=True, stop=True)
                        src = pqk.rearrange("p (pr e) -> p pr e", e=2)
                        d0 = qz0[jt].rearrange(
                            "p (pr c) -> p pr c", c=32)[:, :, h:h + 1]
                        d1 = qz1[jt].rearrange(
                            "p (pr c) -> p pr c", c=32)[:, :, 16 + h:17 + h]
                        if (h + jt) % 2 == 0:
                            nc.vector.tensor_copy(out=d0, in_=src[:, :, 0:1])
                            nc.scalar.copy(out=d1, in_=src[:, :, 1:2])
                        else:
                            nc.scalar.copy(out=d0, in_=src[:, :, 0:1])
                            nc.vector.tensor_copy(out=d1, in_=src[:, :, 1:2])

                # G^T accumulator layout gts[jt][j, h*64+b_local]
                gts = [main.tile([P, NH * 64], adt, tag=f"gts{t}", name=f"gts{t}", bufs=2)
                       for t in range(HT)]

                for grp_i in range(NGRP):
                    g = s * NGRP + grp_i      # global 8-row group index
                    gtiles = [None, None]     # two 2-pair loads per group
                    stage = work.tile([P, N], F32, tag="stage", bufs=2)

                    # transposed group view for the whole 8-row group via
                    # DMA-transpose: gTg[jt][j, (b_loc, n)] (512 cols)
                    gTg = [work.tile([P, 8 * N], adt, tag=f"gTg{t}",
                                     name=f"gTg{t}", bufs=2)
                           for t in range(HT)]
                    gbase = s * (64 * N) + grp_i * 512
                    for t in range(HT):
                        nc.sync.dma_start_transpose(
                            gTg[t], d_grp[gbase:gbase + 512, t * P:(t + 1) * P])

                    for q in range(4):        # 4 pairs in the group
                        p_ = grp_i * 4 + q    # pair index within sub
                        if q % 2 == 0:
                            gt_ = work.tile([P, 2, H], adt, tag="grp", bufs=3)
                            base = s * (64 * N) + (p_ // 2) * 256
                            nc.sync.dma_start(
                                out=gt_,
                                in_=d_grp[base:base + 256, :].rearrange(
                                    "(ph p) j -> p ph j", p=P))
                            gtiles[q // 2] = gt_
                        gtile = gtiles[q // 2]
                        ph = q % 2

                        lg = ps.tile([32, N], F32, tag="lg", bufs=2)
                        for jt in range(HT):
                            cs = p_ * 32
                            nc.tensor.matmul(
                                lg, qz0[jt][:, cs:cs + 32],
                                gTg[jt][:, q * 128:q * 128 + 64],
                                start=(jt == 0), stop=False)
                            nc.tensor.matmul(
                                lg, qz1[jt][:, cs:cs + 32],
                                gTg[jt][:, q * 128 + 64:(q + 1) * 128],
                                start=False, stop=(jt == HT - 1))

                        # scale + mask into stage rows (whole 32-row pair)
                        r0 = q * 32
                        nc.vector.scalar_tensor_tensor(
                            out=stage[r0:r0 + 32, :], in0=lg,
                            scalar=srow_sb[r0:r0 + 32, g:g + 1],
                            in1=mask_sb[r0:r0 + 32, g, :],
                            op0=mybir.AluOpType.mult, op1=mybir.AluOpType.add)

                    # softmax over the whole 8-row group [128, 64]
                    gg = grp_i % 4
                    if gg == 0:
                        attn_st = work.tile([P, 4, N], F32, tag="attn_st",
                                            bufs=2)
                    negmx = work.tile([P, 1], F32, tag="negmx", bufs=2)
                    nc.vector.tensor_reduce(
                        out=negmx, in_=stage, axis=mybir.AxisListType.X,
                        op=mybir.AluOpType.max, negate=True)
                    ex = work.tile([P, N], F32, tag="ex", bufs=2)
                    ssum = work.tile([P, 1], F32, tag="ssum", bufs=2)
                    nc.scalar.activation(
                        out=ex, in_=stage,
                        func=mybir.ActivationFunctionType.Exp,
                        bias=negmx, scale=1.0, accum_out=ssum)
                    rr = work.tile([P, 1], F32, tag="rr", bufs=2)
                    nc.vector.reciprocal(out=rr, in_=ssum)
                    nc.vector.tensor_scalar_mul(attn_st[:, gg, :], ex, rr)
                    att16 = work.tile([P, N], adt, tag="att16", bufs=2)
                    nc.vector.tensor_scalar_mul(att16, ex, rr)
                    if gg == 3:
                        gb = (g - 3) // 4
                        nc.sync.dma_start(
                            out=d_attn.rearrange(
                                "(g p) n -> p g n", p=P)[:, 4 * gb:4 * gb + 4, :],
                            in_=attn_st)

                    # attn^T of group: patful[0:64]=T(att16); dup to [64:128]
                    pat = ps.tile([64, P], adt, tag="pat", bufs=1)
                    nc.tensor.transpose(pat, att16, ident)
                    patful = work.tile([P, P], adt, tag="patful", bufs=2)
                    nc.vector.tensor_copy(out=patful[0:64, :], in_=pat)
                    nc.sync.dma_start(out=patful[64:128, :],
                                      in_=patful[0:64, :])

                    # G pass for the 4 pairs of this group
                    for q in range(4):
                        p_ = grp_i * 4 + q
                        gtile = gtiles[q // 2]
                        ph = q % 2
                        abd = work.tile([P, 32], adt, tag="abd", bufs=2)
                        nc.gpsimd.memset(abd, 0.0)
                        c0 = (2 * q) * 16
                        c1 = (2 * q + 1) * 16
                        nc.vector.tensor_copy(
                            out=abd[0:64, 0:16], in_=patful[0:64, c0:c0 + 16])
                        nc.vector.tensor_copy(
                            out=abd[64:128, 16:32],
                            in_=patful[64:128, c1:c1 + 16])
                        for jt in range(HT):
                            pg = ps.tile([P, 32], F32, tag="pg", bufs=2)
                            nc.tensor.matmul(
                                pg, gtile[:, ph, jt * P:(jt + 1) * P], abd,
                                start=True, stop=True)
                            dst = gts[jt].rearrange(
                                "p (h b) -> p b h", b=64)[:, 2 * p_:2 * p_ + 2, :]
                            src = pg.rearrange("p (b h) -> p b h", h=16)
                            if jt % 2 == 0:
                                nc.scalar.copy(out=dst, in_=src)
                            else:
                                nc.vector.tensor_copy(out=dst, in_=src)

                # ---- ctx per head: ctxT[h//2][(h%2)*64+d, b] ----
                ctxT = [main.tile([P, 64], F32, tag=f"ctxT{t}", name=f"ctxT{t}", bufs=2)
                        for t in range(HT)]
                for h in range(NH):
                    pc = ps.tile([P, 64], F32, tag="pc", bufs=1)
                    for jc in range(HT):
                        nc.tensor.matmul(
                            pc,
                            wvT_sb[jc][:, (h // 2) * P:(h // 2 + 1) * P],
                            gts[jc][:, h * 64:(h + 1) * 64],
                            start=(jc == 0), stop=(jc == HT - 1))
                    off = (h % 2) * 64
                    if h % 2 == 0:
                        nc.vector.tensor_copy(
                            out=ctxT[h // 2][off:off + 64, :],
                            in_=pc[off:off + 64, :])
                    else:
                        nc.scalar.copy(
                            out=ctxT[h // 2][off:off + 64, :],
                            in_=pc[off:off + 64, :])

                # ---- out projection ----
                ostg = work.tile([P, HT, 64], F32, tag="ostg", bufs=2)
                for ot in range(HT):
                    po = ps.tile([P, 64], F32, tag="pc", bufs=1)
                    for oc in range(HT):
                        nc.tensor.matmul(
                            po, woT_sb[oc][:, ot * P:(ot + 1) * P], ctxT[oc],
                            start=(oc == 0), stop=(oc == HT - 1))
                    if ot % 2 == 0:
                        nc.vector.tensor_copy(out=ostg[:, ot, :], in_=po)
                    else:
                        nc.scalar.copy(out=ostg[:, ot, :], in_=po)
                nc.sync.dma_start(
                    out=d_outT.rearrange(
                        "(t p) b -> p t b", p=P)[:, :, s * 64:(s + 1) * 64],
                    in_=ostg)

    if finalize:
        nc.finalize()
    return nc


def host_prep(q_src, group, mask, group_temp, Wq, Wk, Wv, Wo, nsub=4, adt=ADT):
    """Build per-core in_maps from full inputs."""
    npdt = _np_dt(adt)
    BC = nsub * 64
    wqT = np.ascontiguousarray(Wq.T).astype(npdt)
    wk = Wk.astype(npdt)
    wvT = np.ascontiguousarray(Wv.T).astype(npdt)
    woT = np.ascontiguousarray(Wo.T).astype(np.float32)
    srow_full = (1.0 / (math.sqrt(HD) * group_temp.astype(np.float64))).astype(
        np.float32)                                           # [B]
    maskadd_full = np.where(mask, 0.0, -1e9).astype(np.float32)  # [B, N]

    in_maps = []
    for c in range(NCORES):
        r0 = c * BC
        rows = slice(r0, r0 + BC)
        qsrcT = np.ascontiguousarray(q_src[rows].T).astype(npdt)
        grp = np.ascontiguousarray(
            group[rows].reshape(BC * N, H)).astype(npdt)
        ma = np.repeat(maskadd_full[rows, None, :], NH, axis=1).reshape(
            BC * NH, N).astype(np.float32)
        sr = np.repeat(srow_full[rows, None], NH, axis=1).reshape(
            BC * NH, 1).astype(np.float32)
        in_maps.append({
            "qsrcT": qsrcT, "grp": grp, "wqT": wqT, "wk": wk,
            "wvT": wvT, "woT": woT, "maskadd": ma, "srow": sr,
        })
    return in_maps


_prog_cache = {}


def kernel(q_src, group, mask, group_temp, Wq, Wk, Wv, Wo):
    q_src = np.asarray(q_src, dtype=np.float32)
    group = np.asarray(group, dtype=np.float32)
    mask = np.asarray(mask)
    group_temp = np.asarray(group_temp, dtype=np.float32)
    Wq = np.asarray(Wq, dtype=np.float32)
    Wk = np.asarray(Wk, dtype=np.float32)
    Wv = np.asarray(Wv, dtype=np.float32)
    Wo = np.asarray(Wo, dtype=np.float32)

    key = ("prog", 4, str(ADT))
    if key not in _prog_cache:
        _prog_cache[key] = build_program(nsub=4, adt=ADT)
    nc = _prog_cache[key]

    in_maps = host_prep(q_src, group, mask, group_temp, Wq, Wk, Wv, Wo)
    res = run_bass_kernel_spmd(nc, in_maps, core_ids=list(range(NCORES)))

    out = np.concatenate(
        [res.results[c]["outT"].T for c in range(NCORES)], axis=0)
    attn = np.concatenate(
        [res.results[c]["attn"].reshape(256, NH, N) for c in range(NCORES)],
        axis=0)
    return out.astype(np.float32), attn.astype(np.float32)
